# revision 30
# baseline (speedup 1.0000x reference)
"""Trainium2 Bass kernel for T5-style cross-attention, sharded over 8 NeuronCores.

Sharding: tensor-parallel over heads (16 heads -> 2 per core). Each core
computes Q/K/V projections for its 2 heads (full batch), flash-style
attention with multiplicative exp(position_bias), and a partial output
projection against its row-slice of Wo. The host sums the 8 partial
outputs (the unshard step for a row-sharded Wo).

Key design points:
- fp16 datapath on the wire (x/enc/weights/bias/out) halves HBM traffic;
  fp16 keeps the score path accurate (bf16 would round Q/K too hard).
- The additive position bias is applied as a multiplicative exp(bias)
  factor after exp: exp(s+b) = exp(s)*exp(b), exp(b) precomputed on the
  host. The multiply runs on DVE in its 2x (2-byte) mode, so no PE
  identity-matmul bias-add and no PSUM-resident bias.
- attn (exp output) is bf16: scores reach ~40 with no running-max
  subtraction, so fp16 would overflow; bf16 has fp32's exponent range.
- Score matmuls for the two heads (contraction 64) are issued
  back-to-back at base partitions 0/64 -> separate PE row groups run
  them concurrently on hardware.
- Few, large DMAs; input loads issue from SP, bias from GpSimd/SWDGE.
- Deep software pipelining: attn@V of k-group kg flushes after the
  scores of the NEXT group (crossing batch boundaries), the
  normalization chain (recip -> gpsimd partition-broadcast -> ctx mul)
  flushes one k-group later still, and Wo emission for the previous
  q-window is spread one batch per batch-slot. No engine's in-order
  stream head-of-line blocks another's latency chain.
- Projections are zippered into the first q-windows: K/V + Q(window 0)
  for batch bi+1 emit inside q-window 0's batch loop; Q(window 1)
  emits inside q-window 1. Attention for batch 0 starts while the rest
  of phase A (DMA-bound) still streams.
"""

import sys

try:
    import concourse.bass as bass
except ImportError:
    sys.path.insert(0, "/opt/trn_rl_repo")
    import concourse.bass as bass

import numpy as np
import ml_dtypes

import concourse.mybir as mybir
from concourse import bacc
from concourse.tile import TileContext
from concourse.bass_utils import run_bass_kernel_spmd

F32 = mybir.dt.float32
F16 = mybir.dt.float16
BF16 = mybir.dt.bfloat16

_f16 = ml_dtypes.float16 if hasattr(ml_dtypes, "float16") else np.float16

# Problem sizes (hardcoded per spec)
B, NQ, NKV = 4, 2048, 2048
D_MODEL, N_HEADS, D_K = 1024, 16, 64
N_CORES = 8
HPC = N_HEADS // N_CORES          # heads per core = 2
DH = HPC * D_K                    # 128 partition rows of per-core head dims

QW = 256                          # flash q window
KT = 128                          # k tile (partition dim of S^T)
KG = 4                            # k tiles per exp group ([128, KG*QW] psum)


def build_kernel(b=B, nq=NQ, nkv=NKV, d_model=D_MODEL):
    nc = bacc.Bacc("TRN2", target_bir_lowering=False, debug=False,
                   num_devices=N_CORES)

    xT = nc.dram_tensor("xT", [b, d_model, nq], F16, kind="ExternalInput")
    encT = nc.dram_tensor("encT", [b, d_model, nkv], F16, kind="ExternalInput")
    # exp(position_bias), transposed to [h, k, q]
    ebT = nc.dram_tensor("ebT", [HPC, nkv, nq], F16, kind="ExternalInput")
    wq = nc.dram_tensor("wq", [d_model, DH], F16, kind="ExternalInput")
    wk = nc.dram_tensor("wk", [d_model, DH], F16, kind="ExternalInput")
    wv = nc.dram_tensor("wv", [d_model, DH], F16, kind="ExternalInput")
    wo = nc.dram_tensor("wo", [DH, d_model], F16, kind="ExternalInput")
    ident_d = nc.dram_tensor("ident", [128, 128], F16, kind="ExternalInput")
    out = nc.dram_tensor("out", [b, nq, d_model], F16, kind="ExternalOutput")

    n_m = d_model // 128          # model-dim tiles (8)
    pws = 1024                    # projection psum window
    n_pw = nq // pws              # projection windows (2)
    n_qw = nq // QW               # flash q windows (8)
    n_kt = nkv // KT              # k tiles (16)
    n_kg = n_kt // KG             # exp groups (4)

    with TileContext(nc) as tc:
        with (
            tc.tile_pool(name="cst", bufs=1) as cst,
            tc.tile_pool(name="wpool", bufs=1) as wpool,
            tc.tile_pool(name="qkv", bufs=1) as qkv,
            tc.tile_pool(name="actst", bufs=6) as actst,
            tc.tile_pool(name="sbias", bufs=4) as sbias,
            tc.tile_pool(name="sattn", bufs=6) as sattn,
            tc.tile_pool(name="sraw", bufs=4) as sraw,
            tc.tile_pool(name="sctx", bufs=2 * b) as sctx,
            tc.tile_pool(name="vtstage", bufs=2) as vtstage,
            tc.tile_pool(name="sout", bufs=2) as sout,
            tc.tile_pool(name="ssmall", bufs=4) as ssmall,
            tc.tile_pool(name="psbig", bufs=3, space="PSUM") as psbig,
            tc.tile_pool(name="pssmall", bufs=2, space="PSUM") as pssmall,
        ):
            # ---- constants & weights (single batched DMAs) ----
            ident = cst.tile([128, 128], F16, tag="ident")
            nc.sync.dma_start(out=ident, in_=ident_d[:, :])

            wq_sb = wpool.tile([128, n_m * DH], F16, tag="wq")
            wk_sb = wpool.tile([128, n_m * DH], F16, tag="wk")
            wv_sb = wpool.tile([128, n_m * DH], F16, tag="wv")
            nc.sync.dma_start(
                out=wq_sb.rearrange("p (m d) -> p m d", m=n_m),
                in_=wq.rearrange("(m p) d -> p m d", p=128))
            nc.sync.dma_start(
                out=wk_sb.rearrange("p (m d) -> p m d", m=n_m),
                in_=wk.rearrange("(m p) d -> p m d", p=128))
            nc.sync.dma_start(
                out=wv_sb.rearrange("p (m d) -> p m d", m=n_m),
                in_=wv.rearrange("(m p) d -> p m d", p=128))
            wo_sb = wpool.tile([128, d_model], F16, tag="wo")
            nc.sync.dma_start(out=wo_sb, in_=wo[:, :])

            # ---- persistent activation tiles ----
            qT_sb = qkv.tile([128, b * nq], F16, tag="qT")
            kT_sb = qkv.tile([128, b * nkv], F16, tag="kT")
            # per (bi,kt): [h0 V(64) | ones | h1 V(64) | ones] 130-col block
            # bf16 to match the attn (exp output) dtype in the attn@V matmul
            vones = qkv.tile([128, b * n_kt * 130], BF16, tag="vones")
            # all the ones columns sit at 64 + 65*i: one strided memset
            nc.vector.memset(vones[:, 64::65], 1.0)
            # 1-row zero/one rows for PE-side zeroing of the u accumulators
            # (out = zeros^T @ ones streams 512 rows on PE, cheaper than a
            # DVE memset and off the DVE critical path)
            zrow = cst.tile([1, 128], BF16, tag="zrow")
            nc.vector.memset(zrow, 0.0)
            onerow = cst.tile([1, 2 * QW], BF16, tag="onerow")
            nc.vector.memset(onerow, 1.0)

            eb_tiles = {}

            def emit_bias_load(qw):
                q0 = qw * QW
                for h in range(HPC):
                    t = sbias.tile([128, n_kt * QW], F16, tag="bias",
                                   name=f"eb_{qw}_{h}")
                    nc.gpsimd.dma_start(
                        out=t.rearrange("p (t q) -> p t q", t=n_kt),
                        in_=ebT[h, :, q0:q0 + QW]
                        .rearrange("(t p) q -> p t q", p=KT))
                    eb_tiles[(qw, h)] = t

            # ---- phase A (per batch): K/V projections, then Q windows ----
            def emit_kv(bi):
                ets = []
                for mq in range(4):
                    et = actst.tile([128, 2 * nkv], F16, tag="actst",
                                    name=f"et_{bi}_{mq}")
                    nc.sync.dma_start(
                        out=et.rearrange("p (m k) -> p m k", m=2),
                        in_=encT[bi, mq * 256:(mq + 1) * 256, :]
                        .rearrange("(m p) k -> p m k", p=128))
                    ets.append(et)
                for pw in range(n_pw):
                    k_ps = psbig.tile([128, pws], F32, tag="big")
                    v_ps = psbig.tile([128, pws], F32, tag="big")
                    for m in range(n_m):
                        src = ets[m // 2][
                            :, (m % 2) * nkv + pw * pws:
                               (m % 2) * nkv + (pw + 1) * pws]
                        for s in range(pws // 512):
                            nc.tensor.matmul(
                                k_ps[:, s * 512:(s + 1) * 512],
                                wk_sb[:, m * DH:(m + 1) * DH],
                                src[:, s * 512:(s + 1) * 512],
                                start=(m == 0), stop=(m == n_m - 1))
                            nc.tensor.matmul(
                                v_ps[:, s * 512:(s + 1) * 512],
                                wv_sb[:, m * DH:(m + 1) * DH],
                                src[:, s * 512:(s + 1) * 512],
                                start=(m == 0), stop=(m == n_m - 1))
                    # phase-A PSUM->SBUF copies run on ScalarE (idle in A)
                    nc.scalar.copy(
                        kT_sb[:, bi * nkv + pw * pws: bi * nkv + (pw + 1) * pws],
                        k_ps)
                    vt_win = vtstage.tile([128, pws], F16, tag="vtw")
                    nc.scalar.copy(vt_win, v_ps)
                    # V^T -> V tiles via PE transpose; write both heads'
                    # 64-col halves around the prewritten ones columns
                    for s in range(pws // KT):
                        kt = pw * (pws // KT) + s
                        vt_ps = pssmall.tile([128, 128], F16, tag="small",
                                             name=f"vtp_{bi}_{kt}")
                        nc.tensor.transpose(
                            vt_ps, vt_win[:, s * KT:(s + 1) * KT], ident)
                        base = (bi * n_kt + kt) * 130
                        nc.vector.tensor_copy(
                            vones[:, base:base + 130]
                            .rearrange("p (h d) -> p h d", h=2, d=65)[:, :, 0:64],
                            vt_ps.rearrange("p (h d) -> p h d", h=2))

            def emit_q(bi, pw):
                xts = []
                for mh in range(2):
                    xt = actst.tile([128, 4 * pws], F16, tag="actst",
                                    name=f"xt_{bi}_{pw}_{mh}")
                    nc.sync.dma_start(
                        out=xt.rearrange("p (m k) -> p m k", m=4),
                        in_=xT[bi, mh * 512:(mh + 1) * 512,
                               pw * pws:(pw + 1) * pws]
                        .rearrange("(m p) k -> p m k", p=128))
                    xts.append(xt)
                q_ps = psbig.tile([128, pws], F32, tag="big")
                for m in range(n_m):
                    src = xts[m // 4][:, (m % 4) * pws:(m % 4 + 1) * pws]
                    for s in range(pws // 512):
                        nc.tensor.matmul(
                            q_ps[:, s * 512:(s + 1) * 512],
                            wq_sb[:, m * DH:(m + 1) * DH],
                            src[:, s * 512:(s + 1) * 512],
                            start=(m == 0), stop=(m == n_m - 1))
                nc.scalar.copy(
                    qT_sb[:, bi * nq + pw * pws: bi * nq + (pw + 1) * pws],
                    q_ps)

            emit_kv(0)
            emit_bias_load(0)   # after the first K/V loads win DMA slots
            emit_q(0, 0)

            # ---- phase B: flash attention + output projection ----
            def emit_wo_one(pq0, pctx, bi):
                o_sb = sout.tile([128, 2 * d_model], F16, tag="out")
                for qs in range(QW // 128):
                    o_ps = psbig.tile([128, d_model], F32, tag="big",
                                      name=f"ops_{pq0}_{bi}_{qs}")
                    for e in range(d_model // 512):
                        nc.tensor.matmul(
                            o_ps[:, e * 512:(e + 1) * 512],
                            pctx[:, qs * 128:(qs + 1) * 128],
                            wo_sb[:, e * 512:(e + 1) * 512],
                            start=True, stop=True)
                    nc.vector.tensor_copy(
                        o_sb[:, qs * d_model:(qs + 1) * d_model], o_ps)
                nc.sync.dma_start(
                    out=out[bi, pq0:pq0 + QW, :]
                    .rearrange("(t p) e -> p t e", p=128),
                    in_=o_sb.rearrange("p (t e) -> p t e", t=2))

            # deferred work, flushed later to keep every engine stream busy
            attn_pend = None   # (u, bi, kg, attn dict)
            norm_pend = None   # (u, ctx list, bi)

            def flush_attnv():
                nonlocal attn_pend
                if attn_pend is None:
                    return
                u_, bi_, kg_, attn_ = attn_pend
                for h in range(HPC):
                    for j in range(KG):
                        kt = kg_ * KG + j
                        o = (bi_ * n_kt + kt) * 130 + h * 65
                        # start=False always: u is pre-zeroed by memset, and
                        # the two heads' interleaved accumulation sequences
                        # share this PSUM bank (a bank-level start clear
                        # would wipe the other head's partials).
                        nc.tensor.matmul(
                            u_[:, h * QW:(h + 1) * QW],
                            vones[:, o:o + D_K + 1],
                            attn_[h][:, j * QW:(j + 1) * QW],
                            start=False, stop=(kt == n_kt - 1),
                            skip_group_check=True)
                attn_pend = None

            def flush_norm():
                nonlocal norm_pend
                if norm_pend is None:
                    return
                u_, ctx_, bi_ = norm_pend
                recip = ssmall.tile([1, 2 * QW], F32, tag="recip",
                                    name="recip")
                nc.vector.reciprocal(recip, u_[D_K:D_K + 1, :])
                rb = ssmall.tile([D_K, 2 * QW], F32, tag="rb", name="rb")
                nc.gpsimd.partition_broadcast(rb, recip)
                for h in range(HPC):
                    hp = h * D_K
                    with nc.allow_low_precision(reason="f16 ctx for PE"):
                        nc.vector.tensor_mul(
                            ctx_[hp:hp + D_K, :],
                            u_[0:D_K, h * QW:(h + 1) * QW],
                            rb[:, h * QW:(h + 1) * QW])
                norm_pend = None

            prev_qw = None     # (q0, ctx_t) of the previous q-window
            for qw in range(n_qw):
                q0 = qw * QW
                if qw + 1 < n_qw:
                    emit_bias_load(qw + 1)
                ctx_t = [sctx.tile([128, QW], F16, tag="ctx",
                                   name=f"ctx_{qw}_{bi}")
                         for bi in range(b)]
                for bi in range(b):
                    if qw == 0 and bi + 1 < b:
                        emit_kv(bi + 1)
                        emit_q(bi + 1, 0)
                    if qw == 1:
                        emit_q(bi, 1)
                    if prev_qw is not None:
                        emit_wo_one(prev_qw[0], prev_qw[1][bi], bi)
                    u = pssmall.tile([D_K + 1, 2 * QW], F32, tag="small",
                                     name=f"u_{qw}_{bi}")
                    # zero u on the PE (start=True on a fresh bank is safe:
                    # no other accumulation sequence is live in it yet)
                    nc.tensor.matmul(u, zrow[:, 0:D_K + 1], onerow,
                                     start=True, stop=True,
                                     skip_group_check=True)
                    for kg in range(n_kg):
                        s_g = {}
                        for h in range(HPC):
                            s_g[h] = psbig.tile([128, KG * QW], F32,
                                                tag="big", name=f"sg_{h}")
                        for j in range(KG):
                            kt = kg * KG + j
                            # two heads at base partitions 0/64: concurrent
                            # PE row groups on hardware
                            for h in range(HPC):
                                hp = h * D_K
                                nc.tensor.matmul(
                                    s_g[h][:, j * QW:(j + 1) * QW],
                                    kT_sb[hp:hp + D_K,
                                          bi * nkv + kt * KT:
                                          bi * nkv + (kt + 1) * KT],
                                    qT_sb[hp:hp + D_K,
                                          bi * nq + q0: bi * nq + q0 + QW],
                                    start=True, stop=True)
                        if kg == 1:
                            flush_norm()
                        attn = {}
                        for h in range(HPC):
                            raw = sraw.tile([128, KG * QW], BF16,
                                            tag="raw", name=f"raw_{h}")
                            nc.scalar.activation(
                                raw, s_g[h], mybir.ActivationFunctionType.Exp)
                            attn[h] = sattn.tile([128, KG * QW], BF16,
                                                 tag="attn", name=f"attn_{h}")
                            eb = eb_tiles[(qw, h)]
                            # all-SBUF 2-byte multiply -> DVE 2x mode
                            with nc.allow_low_precision(
                                    reason="bf16 attn weights"):
                                nc.vector.tensor_mul(
                                    attn[h], raw,
                                    eb[:, kg * KG * QW:(kg + 1) * KG * QW])
                        flush_attnv()
                        attn_pend = (u, bi, kg, attn)
                    norm_pend = (u, ctx_t[bi], bi)
                prev_qw = (q0, ctx_t)
            flush_attnv()
            flush_norm()
            for bi in range(b):
                emit_wo_one(prev_qw[0], prev_qw[1][bi], bi)
    nc.compile()
    return nc


_NC_CACHE = {}


def _get_nc():
    if "nc" not in _NC_CACHE:
        _NC_CACHE["nc"] = build_kernel()
    return _NC_CACHE["nc"]


def make_in_maps(x, encoding, position_bias, Wq, Wk, Wv, Wo):
    """Shard + transpose + downconvert the full inputs into the 8 cores'
    input maps (all host-side prep; dtypes match the dram declarations)."""
    x = np.asarray(x, np.float32)
    encoding = np.asarray(encoding, np.float32)
    position_bias = np.asarray(position_bias, np.float32)

    xT = np.ascontiguousarray(x.transpose(0, 2, 1)).astype(_f16)
    encT = np.ascontiguousarray(encoding.transpose(0, 2, 1)).astype(_f16)
    eb = np.exp(position_bias[0]).astype(_f16)   # [H, NQ, NKV]
    ident = np.eye(128, dtype=_f16)

    in_maps = []
    for c in range(N_CORES):
        h0 = c * HPC
        in_maps.append({
            "xT": xT,
            "encT": encT,
            "ebT": np.ascontiguousarray(
                eb[h0:h0 + HPC].transpose(0, 2, 1)),
            "wq": np.ascontiguousarray(
                Wq[:, h0 * D_K:(h0 + HPC) * D_K]).astype(_f16),
            "wk": np.ascontiguousarray(
                Wk[:, h0 * D_K:(h0 + HPC) * D_K]).astype(_f16),
            "wv": np.ascontiguousarray(
                Wv[:, h0 * D_K:(h0 + HPC) * D_K]).astype(_f16),
            "wo": np.ascontiguousarray(
                Wo[h0 * D_K:(h0 + HPC) * D_K, :]).astype(_f16),
            "ident": ident,
        })
    return in_maps


def kernel(x, encoding, position_bias, Wq, Wk, Wv, Wo):
    in_maps = make_in_maps(x, encoding, position_bias, Wq, Wk, Wv, Wo)
    nc = _get_nc()
    res = run_bass_kernel_spmd(nc, in_maps, list(range(N_CORES)))
    acc = res.results[0]["out"].astype(np.float32)
    for c in range(1, N_CORES):
        acc = acc + res.results[c]["out"].astype(np.float32)
    return acc


# revision 34
# speedup vs baseline: 1.5030x; 1.5030x over previous
"""Trainium2 Bass kernel for T5-style cross-attention, sharded over 8 NeuronCores.

Sharding: tensor-parallel over heads (16 heads -> 2 per core). Each core
computes Q/K/V projections for its 2 heads (full batch), flash-style
attention with multiplicative exp(position_bias), and a partial output
projection against its row-slice of Wo. The host sums the 8 partial
outputs (the unshard step for a row-sharded Wo).

Key design points:
- fp16 datapath on the wire (x/enc/weights/bias/out) halves HBM traffic;
  fp16 keeps the score path accurate (bf16 would round Q/K too hard).
- The additive position bias is applied as a multiplicative exp(bias)
  factor after exp: exp(s+b) = exp(s)*exp(b), exp(b) precomputed on the
  host. The multiply runs on DVE in its 2x (2-byte) mode, so no PE
  identity-matmul bias-add and no PSUM-resident bias.
- attn (exp output) is bf16: scores reach ~40 with no running-max
  subtraction, so fp16 would overflow; bf16 has fp32's exponent range.
- Score matmuls for the two heads (contraction 64) are issued
  back-to-back at base partitions 0/64 -> separate PE row groups run
  them concurrently on hardware.
- Few, large DMAs; input loads issue from SP, bias from GpSimd/SWDGE.
- Deep software pipelining: attn@V of k-group kg flushes after the
  scores of the NEXT group (crossing batch boundaries), the
  normalization chain (recip -> gpsimd partition-broadcast -> ctx mul)
  flushes one k-group later still, and Wo emission for the previous
  q-window is spread one batch per batch-slot. No engine's in-order
  stream head-of-line blocks another's latency chain.
- Projections are zippered into the first q-windows: K/V + Q(window 0)
  for batch bi+1 emit inside q-window 0's batch loop; Q(window 1)
  emits inside q-window 1. Attention for batch 0 starts while the rest
  of phase A (DMA-bound) still streams.
"""

import sys

try:
    import concourse.bass as bass
except ImportError:
    sys.path.insert(0, "/opt/trn_rl_repo")
    import concourse.bass as bass

import numpy as np
import ml_dtypes

import concourse.mybir as mybir
from concourse import bacc
from concourse.tile import TileContext
from concourse.bass_utils import run_bass_kernel_spmd

F32 = mybir.dt.float32
F16 = mybir.dt.float16
BF16 = mybir.dt.bfloat16

_f16 = ml_dtypes.float16 if hasattr(ml_dtypes, "float16") else np.float16

# Problem sizes (hardcoded per spec)
B, NQ, NKV = 4, 2048, 2048
D_MODEL, N_HEADS, D_K = 1024, 16, 64
N_CORES = 8
HPC = N_HEADS // N_CORES          # heads per core = 2
DH = HPC * D_K                    # 128 partition rows of per-core head dims

QW = 256                          # flash q window
KT = 128                          # k tile (partition dim of S^T)
KG = 4                            # k tiles per exp group ([128, KG*QW] psum)


def build_kernel(b=B, nq=NQ, nkv=NKV, d_model=D_MODEL):
    nc = bacc.Bacc("TRN2", target_bir_lowering=False, debug=False,
                   num_devices=N_CORES)

    xT = nc.dram_tensor("xT", [b, d_model, nq], F16, kind="ExternalInput")
    encT = nc.dram_tensor("encT", [b, d_model, nkv], F16, kind="ExternalInput")
    # exp(position_bias), transposed to [h, k, q]
    ebT = nc.dram_tensor("ebT", [HPC, nkv, nq], F16, kind="ExternalInput")
    wq = nc.dram_tensor("wq", [d_model, DH], F16, kind="ExternalInput")
    wk = nc.dram_tensor("wk", [d_model, DH], F16, kind="ExternalInput")
    wv = nc.dram_tensor("wv", [d_model, DH], F16, kind="ExternalInput")
    wo = nc.dram_tensor("wo", [DH, d_model], F16, kind="ExternalInput")
    ident_d = nc.dram_tensor("ident", [128, 128], F16, kind="ExternalInput")
    out = nc.dram_tensor("out", [b, nq, d_model], F16, kind="ExternalOutput")

    n_m = d_model // 128          # model-dim tiles (8)
    pws = 1024                    # projection psum window
    n_pw = nq // pws              # projection windows (2)
    n_qw = nq // QW               # flash q windows (8)
    n_kt = nkv // KT              # k tiles (16)
    n_kg = n_kt // KG             # exp groups (4)

    with TileContext(nc) as tc:
        with (
            tc.tile_pool(name="cst", bufs=1) as cst,
            tc.tile_pool(name="wpool", bufs=1) as wpool,
            tc.tile_pool(name="qkv", bufs=1) as qkv,
            tc.tile_pool(name="actst", bufs=6) as actst,
            tc.tile_pool(name="sbias", bufs=4) as sbias,
            tc.tile_pool(name="sattn", bufs=6) as sattn,
            tc.tile_pool(name="sraw", bufs=4) as sraw,
            tc.tile_pool(name="sctx", bufs=2 * b) as sctx,
            tc.tile_pool(name="vtstage", bufs=2) as vtstage,
            tc.tile_pool(name="sout", bufs=2) as sout,
            tc.tile_pool(name="ssmall", bufs=4) as ssmall,
            tc.tile_pool(name="psbig", bufs=3, space="PSUM") as psbig,
            tc.tile_pool(name="pssmall", bufs=2, space="PSUM") as pssmall,
        ):
            # ---- constants & weights (single batched DMAs) ----
            ident = cst.tile([128, 128], F16, tag="ident")
            nc.sync.dma_start(out=ident, in_=ident_d[:, :])

            wq_sb = wpool.tile([128, n_m * DH], F16, tag="wq")
            wk_sb = wpool.tile([128, n_m * DH], F16, tag="wk")
            wv_sb = wpool.tile([128, n_m * DH], F16, tag="wv")
            nc.sync.dma_start(
                out=wq_sb.rearrange("p (m d) -> p m d", m=n_m),
                in_=wq.rearrange("(m p) d -> p m d", p=128))
            nc.sync.dma_start(
                out=wk_sb.rearrange("p (m d) -> p m d", m=n_m),
                in_=wk.rearrange("(m p) d -> p m d", p=128))
            nc.sync.dma_start(
                out=wv_sb.rearrange("p (m d) -> p m d", m=n_m),
                in_=wv.rearrange("(m p) d -> p m d", p=128))
            wo_sb = wpool.tile([128, d_model], F16, tag="wo")
            nc.sync.dma_start(out=wo_sb, in_=wo[:, :])

            # ---- persistent activation tiles ----
            qT_sb = qkv.tile([128, b * nq], F16, tag="qT")
            kT_sb = qkv.tile([128, b * nkv], F16, tag="kT")
            # per (bi,kt): [h0 V(64) | ones | h1 V(64) | ones] 130-col block
            # bf16 to match the attn (exp output) dtype in the attn@V matmul
            vones = qkv.tile([128, b * n_kt * 130], BF16, tag="vones")
            # all the ones columns sit at 64 + 65*i: one strided memset
            nc.vector.memset(vones[:, 64::65], 1.0)
            # 1-row zero/one rows for PE-side zeroing of the u accumulators
            # (out = zeros^T @ ones streams 512 rows on PE, cheaper than a
            # DVE memset and off the DVE critical path)
            zrow = cst.tile([1, 128], BF16, tag="zrow")
            nc.vector.memset(zrow, 0.0)
            onerow = cst.tile([1, 2 * QW], BF16, tag="onerow")
            nc.vector.memset(onerow, 1.0)

            eb_tiles = {}

            def emit_bias_load(qw):
                q0 = qw * QW
                for h in range(HPC):
                    t = sbias.tile([128, n_kt * QW], F16, tag="bias",
                                   name=f"eb_{qw}_{h}")
                    nc.gpsimd.dma_start(
                        out=t.rearrange("p (t q) -> p t q", t=n_kt),
                        in_=ebT[h, :, q0:q0 + QW]
                        .rearrange("(t p) q -> p t q", p=KT))
                    eb_tiles[(qw, h)] = t

            # ---- phase A (per batch): K/V projections, then Q windows ----
            def emit_kv(bi):
                ets = []
                for mq in range(4):
                    et = actst.tile([128, 2 * nkv], F16, tag="actst",
                                    name=f"et_{bi}_{mq}")
                    nc.sync.dma_start(
                        out=et.rearrange("p (m k) -> p m k", m=2),
                        in_=encT[bi, mq * 256:(mq + 1) * 256, :]
                        .rearrange("(m p) k -> p m k", p=128))
                    ets.append(et)
                for pw in range(n_pw):
                    k_ps = psbig.tile([128, pws], F32, tag="big")
                    v_ps = psbig.tile([128, pws], F32, tag="big")
                    for m in range(n_m):
                        src = ets[m // 2][
                            :, (m % 2) * nkv + pw * pws:
                               (m % 2) * nkv + (pw + 1) * pws]
                        for s in range(pws // 512):
                            nc.tensor.matmul(
                                k_ps[:, s * 512:(s + 1) * 512],
                                wk_sb[:, m * DH:(m + 1) * DH],
                                src[:, s * 512:(s + 1) * 512],
                                start=(m == 0), stop=(m == n_m - 1))
                            nc.tensor.matmul(
                                v_ps[:, s * 512:(s + 1) * 512],
                                wv_sb[:, m * DH:(m + 1) * DH],
                                src[:, s * 512:(s + 1) * 512],
                                start=(m == 0), stop=(m == n_m - 1))
                    # phase-A PSUM->SBUF copies run on ScalarE (idle in A)
                    nc.scalar.copy(
                        kT_sb[:, bi * nkv + pw * pws: bi * nkv + (pw + 1) * pws],
                        k_ps)
                    vt_win = vtstage.tile([128, pws], F16, tag="vtw")
                    nc.scalar.copy(vt_win, v_ps)
                    # V^T -> V tiles via PE transpose; write both heads'
                    # 64-col halves around the prewritten ones columns
                    for s in range(pws // KT):
                        kt = pw * (pws // KT) + s
                        vt_ps = pssmall.tile([128, 128], F16, tag="small",
                                             name=f"vtp_{bi}_{kt}")
                        nc.tensor.transpose(
                            vt_ps, vt_win[:, s * KT:(s + 1) * KT], ident)
                        base = (bi * n_kt + kt) * 130
                        nc.vector.tensor_copy(
                            vones[:, base:base + 130]
                            .rearrange("p (h d) -> p h d", h=2, d=65)[:, :, 0:64],
                            vt_ps.rearrange("p (h d) -> p h d", h=2))

            def emit_q(bi, pw):
                xts = []
                for mh in range(2):
                    xt = actst.tile([128, 4 * pws], F16, tag="actst",
                                    name=f"xt_{bi}_{pw}_{mh}")
                    nc.sync.dma_start(
                        out=xt.rearrange("p (m k) -> p m k", m=4),
                        in_=xT[bi, mh * 512:(mh + 1) * 512,
                               pw * pws:(pw + 1) * pws]
                        .rearrange("(m p) k -> p m k", p=128))
                    xts.append(xt)
                q_ps = psbig.tile([128, pws], F32, tag="big")
                for m in range(n_m):
                    src = xts[m // 4][:, (m % 4) * pws:(m % 4 + 1) * pws]
                    for s in range(pws // 512):
                        nc.tensor.matmul(
                            q_ps[:, s * 512:(s + 1) * 512],
                            wq_sb[:, m * DH:(m + 1) * DH],
                            src[:, s * 512:(s + 1) * 512],
                            start=(m == 0), stop=(m == n_m - 1))
                nc.scalar.copy(
                    qT_sb[:, bi * nq + pw * pws: bi * nq + (pw + 1) * pws],
                    q_ps)

            emit_kv(0)
            emit_bias_load(0)   # after the first K/V loads win DMA slots
            emit_q(0, 0)

            # ---- phase B: flash attention + output projection ----
            def emit_wo_one(pq0, pctx, bi):
                o_sb = sout.tile([128, 2 * d_model], F16, tag="out")
                for qs in range(QW // 128):
                    o_ps = psbig.tile([128, d_model], F32, tag="big",
                                      name=f"ops_{pq0}_{bi}_{qs}")
                    for e in range(d_model // 512):
                        nc.tensor.matmul(
                            o_ps[:, e * 512:(e + 1) * 512],
                            pctx[:, qs * 128:(qs + 1) * 128],
                            wo_sb[:, e * 512:(e + 1) * 512],
                            start=True, stop=True)
                    nc.vector.tensor_copy(
                        o_sb[:, qs * d_model:(qs + 1) * d_model], o_ps)
                nc.sync.dma_start(
                    out=out[bi, pq0:pq0 + QW, :]
                    .rearrange("(t p) e -> p t e", p=128),
                    in_=o_sb.rearrange("p (t e) -> p t e", t=2))

            # deferred work, flushed later to keep every engine stream busy
            attn_pend = None   # (u, bi, kg, attn dict)
            norm_pend = None   # (u, ctx list, bi)

            def flush_attnv():
                nonlocal attn_pend
                if attn_pend is None:
                    return
                u_, bi_, kg_, attn_ = attn_pend
                for h in range(HPC):
                    for j in range(KG):
                        kt = kg_ * KG + j
                        o = (bi_ * n_kt + kt) * 130 + h * 65
                        # start=False always: u is pre-zeroed by memset, and
                        # the two heads' interleaved accumulation sequences
                        # share this PSUM bank (a bank-level start clear
                        # would wipe the other head's partials).
                        nc.tensor.matmul(
                            u_[:, h * QW:(h + 1) * QW],
                            vones[:, o:o + D_K + 1],
                            attn_[h][:, j * QW:(j + 1) * QW],
                            start=False, stop=(kt == n_kt - 1),
                            skip_group_check=True)
                attn_pend = None

            def flush_norm():
                nonlocal norm_pend
                if norm_pend is None:
                    return
                u_, ctx_, bi_ = norm_pend
                recip = ssmall.tile([1, 2 * QW], F32, tag="recip",
                                    name="recip")
                nc.vector.reciprocal(recip, u_[D_K:D_K + 1, :])
                rb = ssmall.tile([D_K, 2 * QW], F32, tag="rb", name="rb")
                nc.gpsimd.partition_broadcast(rb, recip)
                for h in range(HPC):
                    hp = h * D_K
                    with nc.allow_low_precision(reason="f16 ctx for PE"):
                        nc.vector.tensor_mul(
                            ctx_[hp:hp + D_K, :],
                            u_[0:D_K, h * QW:(h + 1) * QW],
                            rb[:, h * QW:(h + 1) * QW])
                norm_pend = None

            prev_qw = None     # (q0, ctx_t) of the previous q-window
            for qw in range(n_qw):
                q0 = qw * QW
                if qw + 1 < n_qw:
                    emit_bias_load(qw + 1)
                ctx_t = [sctx.tile([128, QW], F16, tag="ctx",
                                   name=f"ctx_{qw}_{bi}")
                         for bi in range(b)]
                for bi in range(b):
                    if qw == 0 and bi + 1 < b:
                        emit_kv(bi + 1)
                        emit_q(bi + 1, 0)
                    if qw == 1:
                        emit_q(bi, 1)
                    if prev_qw is not None:
                        emit_wo_one(prev_qw[0], prev_qw[1][bi], bi)
                    u = pssmall.tile([D_K + 1, 2 * QW], F32, tag="small",
                                     name=f"u_{qw}_{bi}")
                    # zero u on the PE (start=True on a fresh bank is safe:
                    # no other accumulation sequence is live in it yet)
                    nc.tensor.matmul(u, zrow[:, 0:D_K + 1], onerow,
                                     start=True, stop=True,
                                     skip_group_check=True)
                    for kg in range(n_kg):
                        s_g = {}
                        for h in range(HPC):
                            s_g[h] = psbig.tile([128, KG * QW], F32,
                                                tag="big", name=f"sg_{h}")
                        for j in range(KG):
                            kt = kg * KG + j
                            # two heads at base partitions 0/64: concurrent
                            # PE row groups on hardware
                            for h in range(HPC):
                                hp = h * D_K
                                nc.tensor.matmul(
                                    s_g[h][:, j * QW:(j + 1) * QW],
                                    kT_sb[hp:hp + D_K,
                                          bi * nkv + kt * KT:
                                          bi * nkv + (kt + 1) * KT],
                                    qT_sb[hp:hp + D_K,
                                          bi * nq + q0: bi * nq + q0 + QW],
                                    start=True, stop=True)
                        if kg == 1:
                            flush_norm()
                        attn = {}
                        for h in range(HPC):
                            raw = sraw.tile([128, KG * QW], BF16,
                                            tag="raw", name=f"raw_{h}")
                            nc.scalar.activation(
                                raw, s_g[h], mybir.ActivationFunctionType.Exp)
                            attn[h] = sattn.tile([128, KG * QW], BF16,
                                                 tag="attn", name=f"attn_{h}")
                            eb = eb_tiles[(qw, h)]
                            # all-SBUF 2-byte multiply -> DVE 2x mode
                            with nc.allow_low_precision(
                                    reason="bf16 attn weights"):
                                nc.vector.tensor_mul(
                                    attn[h], raw,
                                    eb[:, kg * KG * QW:(kg + 1) * KG * QW])
                        flush_attnv()
                        attn_pend = (u, bi, kg, attn)
                    norm_pend = (u, ctx_t[bi], bi)
                prev_qw = (q0, ctx_t)
            flush_attnv()
            flush_norm()
            for bi in range(b):
                emit_wo_one(prev_qw[0], prev_qw[1][bi], bi)
    nc.compile()
    return nc


_NC_CACHE = {}


def _get_nc():
    if "nc" not in _NC_CACHE:
        _NC_CACHE["nc"] = build_kernel()
    return _NC_CACHE["nc"]


def make_in_maps(x, encoding, position_bias, Wq, Wk, Wv, Wo):
    """Shard + transpose + downconvert the full inputs into the 8 cores'
    input maps (all host-side prep; dtypes match the dram declarations)."""
    x = np.asarray(x, np.float32)
    encoding = np.asarray(encoding, np.float32)
    position_bias = np.asarray(position_bias, np.float32)

    xT = np.ascontiguousarray(x.transpose(0, 2, 1)).astype(_f16)
    encT = np.ascontiguousarray(encoding.transpose(0, 2, 1)).astype(_f16)
    eb = np.exp(position_bias[0]).astype(_f16)   # [H, NQ, NKV]
    ident = np.eye(128, dtype=_f16)

    in_maps = []
    for c in range(N_CORES):
        h0 = c * HPC
        in_maps.append({
            "xT": xT,
            "encT": encT,
            "ebT": np.ascontiguousarray(
                eb[h0:h0 + HPC].transpose(0, 2, 1)),
            "wq": np.ascontiguousarray(
                Wq[:, h0 * D_K:(h0 + HPC) * D_K]).astype(_f16),
            "wk": np.ascontiguousarray(
                Wk[:, h0 * D_K:(h0 + HPC) * D_K]).astype(_f16),
            "wv": np.ascontiguousarray(
                Wv[:, h0 * D_K:(h0 + HPC) * D_K]).astype(_f16),
            "wo": np.ascontiguousarray(
                Wo[h0 * D_K:(h0 + HPC) * D_K, :]).astype(_f16),
            "ident": ident,
        })
    return in_maps


def kernel(x, encoding, position_bias, Wq, Wk, Wv, Wo):
    in_maps = make_in_maps(x, encoding, position_bias, Wq, Wk, Wv, Wo)
    nc = _get_nc()
    res = run_bass_kernel_spmd(nc, in_maps, list(range(N_CORES)))
    acc = res.results[0]["out"].astype(np.float32)
    for c in range(1, N_CORES):
        acc = acc + res.results[c]["out"].astype(np.float32)
    return acc
